# revision 1
# baseline (speedup 1.0000x reference)
"""Trainium2 Bass kernel for cosine-similarity multi-head attention.

Math (per batch element b):
    context = query @ w_q.T + b_q                    # [S, 120]
    ctx     = context * weight_tensor                # bcast [1,120]
    ctx_n   = ctx / max(||ctx||_2(axis=-1), 1e-12)   # L2 normalize
    scores  = ctx_n @ ctx_n.T                        # [S, S]
    out     = softmax(where(mask==0, -1e9, scores))  # row softmax

Sharding: data-parallel over batch. 8 batch elements -> 8 NeuronCores,
each core computes its own [S, S] output tile independently.

Kernel structure per core:
  Phase 0: load + transpose w_q (fold weight_tensor/bias in), identity.
  Phase 1: per 128-row s-tile: transpose query via PE, project to
           context, L2-normalize rows (sqrt + Newton refine), transpose
           back into a persistent ctxT [120, S] SBUF tensor.
  Phase 2: per 128-row q-tile: PE matmul scores chunks -> ACT exp ->
           DVE tensor_tensor_reduce (mask multiply + row-sum fused) ->
           reciprocal -> scale -> DMA out.  Softmax skips the row-max
           subtraction: scores are cosine similarities in [-1, 1], and
           masked entries are exactly zeroed by the mask multiply.
"""

import sys

if "/opt/trn_rl_repo" not in sys.path:
    sys.path.insert(0, "/opt/trn_rl_repo")

from contextlib import ExitStack

import numpy as np

import concourse.bass as bass
import concourse.mybir as mybir
import concourse.tile as tile
from concourse import bacc
from concourse.dve_ops import TENSOR_TENSOR_REDUCE as TTR_OP
from concourse.masks import make_identity

D_MODEL = 512
H_DIM = 120
N_CORES = 8
P = 128  # partition tile

F32 = mybir.dt.float32
I32 = mybir.dt.int32
Alu = mybir.AluOpType
Act = mybir.ActivationFunctionType

CFG = dict(
    chunk=1024,      # phase-2 column chunk (multiple of 512)
    mask_bufs=9,     # deep prefetch; mask tile doubles as the output buffer
    ech_bufs=4,      # small exp-chunk temps [128, chunk]
    ps2_bufs=3,      # phase-2 psum tiles [128, chunk]
    dma_split=4096,  # column width per dma_start for mask/out
    scores_f32r=True,  # float32r (tf32-like) for the big SxS matmul
)


def build_nc(S: int = 4096):
    nc = bacc.Bacc("TRN2", target_bir_lowering=False, debug=False)

    q_dram = nc.dram_tensor("query", [S, D_MODEL], F32, kind="ExternalInput")
    m_dram = nc.dram_tensor("mask", [S, S], I32, kind="ExternalInput")
    wq_dram = nc.dram_tensor("w_q", [H_DIM, D_MODEL], F32, kind="ExternalInput")
    bq_dram = nc.dram_tensor("b_q", [H_DIM], F32, kind="ExternalInput")
    wt_dram = nc.dram_tensor("weight_tensor", [1, H_DIM], F32, kind="ExternalInput")
    out_dram = nc.dram_tensor("out", [S, S], F32, kind="ExternalOutput")

    NT = S // P                      # 128-row tiles
    CHUNK = min(CFG["chunk"], S)
    NCH = S // CHUNK
    DSP = min(CFG["dma_split"], S)   # dma column split
    NDS = S // DSP
    ND = D_MODEL // P                # 4 chunks of contraction dim

    with tile.TileContext(nc) as tc, ExitStack() as ctx:
        singles = ctx.enter_context(tc.tile_pool(name="singles", bufs=1))

        # ---------- Phase 0: constants ----------
        ident = singles.tile([P, P], F32)
        make_identity(nc, ident)

        # weight_tensor broadcast to all 128 partitions: [128, 120]
        wtb = singles.tile([P, H_DIM], F32)
        nc.gpsimd.dma_start(
            out=wtb,
            in_=bass.AP(tensor=wt_dram, offset=0, ap=[[0, P], [1, H_DIM]]),
        )

        # b_q * weight_tensor -> bw [1, 120]
        bq_sb = singles.tile([1, H_DIM], F32)
        nc.gpsimd.dma_start(
            out=bq_sb,
            in_=bass.AP(tensor=bq_dram, offset=0, ap=[[0, 1], [1, H_DIM]]),
        )
        bw = singles.tile([1, H_DIM], F32)
        nc.vector.tensor_mul(bw, bq_sb, wtb[:1, :])

        ones_row = singles.tile([1, P], F32)
        nc.vector.memset(ones_row, 1.0)

        # w_q [120, 512] -> transposed+scaled wqTs [4x128, 120]
        wq_sb = singles.tile([H_DIM, D_MODEL], F32)
        nc.sync.dma_start(out=wq_sb, in_=wq_dram.ap())
        wqTs = singles.tile([P, ND * H_DIM], F32)

        # persistent normalized-transposed context [120 (pad 128), S].
        # Stored as float32r so the SxS matmul runs at 1 cycle/row (4x fp32);
        # the eviction copies below perform the f32 -> f32r rounding.
        ctxT = singles.tile([P, S], mybir.dt.float32r if CFG["scores_f32r"]
                            else F32)

        with ExitStack() as ph0:
            ps_w = ph0.enter_context(
                tc.tile_pool(name="ps_w", bufs=2, space="PSUM"))
            for c in range(ND):
                wqT_ps = ps_w.tile([P, H_DIM], F32)
                nc.tensor.transpose(
                    wqT_ps, wq_sb[:, c * P:(c + 1) * P], ident[:H_DIM, :H_DIM])
                # evict + fold in weight_tensor scale
                nc.vector.tensor_mul(
                    wqTs[:, c * H_DIM:(c + 1) * H_DIM], wqT_ps, wtb)

        with ExitStack() as ph1:
            # ---------- Phase 1: build ctxT ----------
            qin_p = ph1.enter_context(tc.tile_pool(name="qin", bufs=4))
            qt_p = ph1.enter_context(tc.tile_pool(name="qt", bufs=2))
            tmp_p = ph1.enter_context(tc.tile_pool(name="ph1tmp", bufs=2))
            st_p = ph1.enter_context(tc.tile_pool(name="ph1st", bufs=2))
            ps_t = ph1.enter_context(
                tc.tile_pool(name="ps_t", bufs=3, space="PSUM"))
            ps_c = ph1.enter_context(
                tc.tile_pool(name="ps_c", bufs=2, space="PSUM"))
            ps_ct = ph1.enter_context(
                tc.tile_pool(name="ps_ct", bufs=2, space="PSUM"))

            for i in range(NT):
                s0 = i * P
                # query via SWDGE so the sync ring is free for mask prefetch
                q_in = qin_p.tile([P, D_MODEL], F32)
                nc.gpsimd.dma_start(out=q_in, in_=q_dram[s0:s0 + P, :])

                # transpose query tile -> qT [d, s] chunks
                qT = qt_p.tile([P, D_MODEL], F32)
                for c in range(ND):
                    tp = ps_t.tile([P, P], F32, tag="tp")
                    nc.tensor.transpose(tp, q_in[:, c * P:(c + 1) * P], ident)
                    eng = nc.vector if c % 2 == 0 else nc.scalar
                    if eng is nc.vector:
                        nc.vector.tensor_copy(qT[:, c * P:(c + 1) * P], tp)
                    else:
                        nc.scalar.copy(qT[:, c * P:(c + 1) * P], tp)

                # context tile [s=128, k=120] = q @ (w_q * wt).T + b*wt
                ctx_ps = ps_c.tile([P, H_DIM], F32)
                for c in range(ND):
                    nc.tensor.matmul(
                        ctx_ps,
                        lhsT=qT[:, c * P:(c + 1) * P],
                        rhs=wqTs[:, c * H_DIM:(c + 1) * H_DIM],
                        start=(c == 0), stop=False)
                nc.tensor.matmul(
                    ctx_ps, lhsT=ones_row, rhs=bw, start=False, stop=True)

                # row L2 norm^2: ACT Square with free-dim accumulate
                sq = tmp_p.tile([P, H_DIM], F32, tag="sq")
                nsq = st_p.tile([P, 1], F32, tag="nsq")
                nc.scalar.activation(sq, ctx_ps, Act.Square, accum_out=nsq)

                # norm = sqrt(nsq), one Newton step, then rstd = 2/(s0+x/s0)
                sroot = st_p.tile([P, 1], F32, tag="sroot")
                nc.scalar.activation(sroot, nsq, Act.Sqrt)
                r0 = st_p.tile([P, 1], F32, tag="r0")
                nc.vector.reciprocal(r0, sroot)
                t1 = st_p.tile([P, 1], F32, tag="t1")
                nc.vector.tensor_mul(t1, nsq, r0)
                ssum = st_p.tile([P, 1], F32, tag="ssum")
                nc.vector.tensor_add(ssum, sroot, t1)
                nc.vector.tensor_scalar_max(ssum, ssum, 2e-12)
                u = st_p.tile([P, 1], F32, tag="u")
                nc.vector.reciprocal(u, ssum)
                rstd = st_p.tile([P, 1], F32, tag="rstd")
                nc.vector.tensor_scalar_mul(rstd, u, 2.0)

                # normalize + evict: ctx_n [s, k]
                ctxn = tmp_p.tile([P, H_DIM], F32, tag="ctxn")
                nc.scalar.activation(ctxn, ctx_ps, Act.Copy, scale=rstd)

                # transpose to [k, s] and park into ctxT
                ctxT_ps = ps_ct.tile([H_DIM, P], F32)
                nc.tensor.transpose(ctxT_ps, ctxn, ident)
                if i % 2 == 0:
                    nc.vector.tensor_copy(ctxT[:H_DIM, s0:s0 + P], ctxT_ps)
                else:
                    nc.scalar.copy(ctxT[:H_DIM, s0:s0 + P], ctxT_ps)

        # ---------- Phase 2: scores + masked softmax ----------
        with ExitStack() as ph2:
            mask_p = ph2.enter_context(
                tc.tile_pool(name="maskp", bufs=CFG["mask_bufs"]))
            ech_p = ph2.enter_context(
                tc.tile_pool(name="echp", bufs=CFG["ech_bufs"]))
            sum_p = ph2.enter_context(tc.tile_pool(name="sump", bufs=3))
            ps2 = ph2.enter_context(
                tc.tile_pool(name="ps2", bufs=CFG["ps2_bufs"], space="PSUM"))

            for i in range(NT):
                q0 = i * P
                mask_sb = mask_p.tile([P, S], I32)
                for d in range(NDS):
                    nc.sync.dma_start(
                        out=mask_sb[:, d * DSP:(d + 1) * DSP],
                        in_=m_dram[q0:q0 + P, d * DSP:(d + 1) * DSP])
                # f32 view of the same bytes: masked exp overwrites the mask
                # tile in place, so one 16KB/partition pool serves mask in,
                # softmax scratch, and the store buffer.
                maskf = mask_sb.bitcast(F32)

                sums = sum_p.tile([P, NCH], F32, tag="sums")
                lhsT = ctxT[:H_DIM, q0:q0 + P]
                for j in range(NCH):
                    c0 = j * CHUNK
                    sc_ps = ps2.tile([P, CHUNK], F32)
                    for h in range(CHUNK // 512):
                        nc.tensor.matmul(
                            sc_ps[:, h * 512:(h + 1) * 512],
                            lhsT=lhsT,
                            rhs=ctxT[:H_DIM, c0 + h * 512:c0 + (h + 1) * 512],
                            start=True, stop=True)
                    # exp (scores in [-1, 1]; masked entries zeroed next)
                    ech = ech_p.tile([P, CHUNK], F32)
                    nc.scalar.activation(ech, sc_ps, Act.Exp)
                    # fused mask-multiply + row-sum (chained across chunks);
                    # custom-DVE uop: out = in0*in1*s1, accum = s0 + sum(out)
                    nc.vector._custom_dve(
                        TTR_OP,
                        out=maskf[:, c0:c0 + CHUNK],
                        in0=ech,
                        in1=mask_sb[:, c0:c0 + CHUNK],
                        s0=(0.0 if j == 0 else sums[:, j - 1:j]),
                        s1=1.0,
                        accum_out=sums[:, j:j + 1])

                rden = sum_p.tile([P, 1], F32, tag="rden")
                nc.vector.reciprocal(rden, sums[:, NCH - 1:NCH])

                # normalize in place, alternating engines, then store via the
                # ACT HWDGE ring (so blocked stores can't head-of-line-block
                # mask prefetch on the sync ring)
                for j in range(NCH):
                    c0 = j * CHUNK
                    if j % 2 == 0:
                        nc.scalar.activation(
                            maskf[:, c0:c0 + CHUNK], maskf[:, c0:c0 + CHUNK],
                            Act.Copy, scale=rden)
                    else:
                        nc.vector.tensor_scalar_mul(
                            maskf[:, c0:c0 + CHUNK], maskf[:, c0:c0 + CHUNK],
                            rden)
                for d in range(NDS):
                    nc.scalar.dma_start(
                        out=out_dram[q0:q0 + P, d * DSP:(d + 1) * DSP],
                        in_=maskf[:, d * DSP:(d + 1) * DSP])

    nc.compile()
    return nc


def _run(nc, in_maps, trace=False, tmpdir=None):
    from concourse import bass_utils
    return bass_utils.run_bass_kernel_spmd(
        nc, in_maps, core_ids=list(range(len(in_maps))), trace=trace,
        tmpdir=tmpdir)


def kernel(**inputs: np.ndarray) -> np.ndarray:
    query = np.ascontiguousarray(np.asarray(inputs["query"], np.float32))
    mask = np.ascontiguousarray(np.asarray(inputs["mask"], np.int32))
    w_q = np.ascontiguousarray(np.asarray(inputs["w_q"], np.float32))
    b_q = np.ascontiguousarray(np.asarray(inputs["b_q"], np.float32))
    wt = np.ascontiguousarray(
        np.asarray(inputs["weight_tensor"], np.float32).reshape(1, H_DIM))

    B, S, _ = query.shape
    assert B == N_CORES
    nc = build_nc(S)
    in_maps = [
        dict(query=query[b], mask=mask[b], w_q=w_q, b_q=b_q, weight_tensor=wt)
        for b in range(B)
    ]
    res = _run(nc, in_maps)
    return np.stack([res.results[b]["out"] for b in range(B)], axis=0)



# revision 4
# speedup vs baseline: 1.5948x; 1.5948x over previous
"""Trainium2 Bass kernel for cosine-similarity multi-head attention (v2).

Math (per batch element b):
    context = query @ w_q.T + b_q                    # [S, 120]
    ctx     = context * weight_tensor                # bcast [1,120]
    ctx_n   = ctx / max(||ctx||_2(axis=-1), 1e-12)   # L2 normalize
    scores  = ctx_n @ ctx_n.T                        # [S, S]
    out     = softmax(where(mask==0, -1e9, scores))  # row softmax

Sharding: data-parallel over batch. 8 batch elements -> 8 NeuronCores.

v2 vs v1 (436us): the v1 kernel was HBM-bound (136MB/core: i32 mask in,
f32 out, f32 query). v2 cuts traffic to ~55MB/core:
  - mask sent as uint8 {0,1}  (64MB -> 16.8MB)
  - output stored as bf16     (64MB -> 33.6MB); well within the 2e-2 gate
  - query sent pre-transposed bf16 [512, S] (8.4 -> 4.2MB), which also
    deletes all 128 PE transposes + PSUM evictions from phase 1
and rebalances the engines:
  - scores matmul in bf16 (1 cy/row, faster weight loads than f32r)
  - ACT does exp only (PSUM -> SBUF bf16), with the masked row-sum done
    by the native ISA tensor_tensor_reduce on DVE (SBUF-only operands ->
    2x DVE mode), replacing the 1x-speed custom-DVE op of v1
  - final 1/sum scale on DVE in bf16
"""

import sys

if "/opt/trn_rl_repo" not in sys.path:
    sys.path.insert(0, "/opt/trn_rl_repo")

from contextlib import ExitStack

import numpy as np
import ml_dtypes

import concourse.bass as bass
import concourse.mybir as mybir
import concourse.tile as tile
from concourse import bacc
from concourse.masks import make_identity

D_MODEL = 512
H_DIM = 120
N_CORES = 8
P = 128  # partition tile

F32 = mybir.dt.float32
BF16 = mybir.dt.bfloat16
U8 = mybir.dt.uint8
Alu = mybir.AluOpType
Act = mybir.ActivationFunctionType

CFG = dict(
    mask_bufs=8,     # u8 mask tiles [128, S] (4KB/part each)
    out_bufs=4,      # bf16 store tiles [128, S] (8KB/part each)
    ech_bufs=6,      # bf16 exp chunks [128, 512]
    ps2_bufs=6,      # phase-2 psum tiles [128, 512] f32 (1 bank each)
    qt_col_split=4,  # qT DMA column segments (earlier phase-1 start)
    scale_split=2,   # scale/store column chunks per tile
)


def build_nc(S: int = 4096, add_bias: bool = False):
    nc = bacc.Bacc("TRN2", target_bir_lowering=False, debug=False)

    qT_dram = nc.dram_tensor("qT", [D_MODEL, S], BF16, kind="ExternalInput")
    m_dram = nc.dram_tensor("mask", [S, S], U8, kind="ExternalInput")
    wq_dram = nc.dram_tensor("w_q", [H_DIM, D_MODEL], BF16, kind="ExternalInput")
    wt_dram = nc.dram_tensor("weight_tensor", [1, H_DIM], F32, kind="ExternalInput")
    if add_bias:
        bq_dram = nc.dram_tensor("b_q", [H_DIM], F32, kind="ExternalInput")
    out_dram = nc.dram_tensor("out", [S, S], BF16, kind="ExternalOutput")

    NT = S // P                      # 128-row tiles
    NCH = S // 512                   # phase-2 psum chunks per tile
    ND = D_MODEL // P                # 4 chunks of contraction dim
    QSEG = S // CFG["qt_col_split"]
    SSP = S // CFG["scale_split"]

    with tile.TileContext(nc) as tc, ExitStack() as ctx:
        singles = ctx.enter_context(tc.tile_pool(name="singles", bufs=1))

        ident = singles.tile([P, P], BF16)
        make_identity(nc, ident)

        # weight_tensor broadcast to all 128 partitions: [128, 120] f32
        wtb = singles.tile([P, H_DIM], F32)
        nc.gpsimd.dma_start(
            out=wtb,
            in_=bass.AP(tensor=wt_dram, offset=0, ap=[[0, P], [1, H_DIM]]),
        )

        # w_q bf16 [120, 512]
        wq_sb = singles.tile([H_DIM, D_MODEL], BF16)
        nc.sync.dma_start(out=wq_sb, in_=wq_dram.ap())

        # resident transposed query, 4 part-tiles of [128, S] bf16.
        # Column-segmented DMA so early phase-1 tiles don't wait on the
        # whole 4.2MB.
        qT = [singles.tile([P, S], BF16, name=f"qT{c}") for c in range(ND)]
        for seg in range(CFG["qt_col_split"]):
            for c in range(ND):
                nc.gpsimd.dma_start(
                    out=qT[c][:, seg * QSEG:(seg + 1) * QSEG],
                    in_=qT_dram[c * P:(c + 1) * P,
                                seg * QSEG:(seg + 1) * QSEG])

        if add_bias:
            bq_sb = singles.tile([1, H_DIM], F32)
            nc.sync.dma_start(
                out=bq_sb,
                in_=bass.AP(tensor=bq_dram, offset=0, ap=[[0, 1], [1, H_DIM]]),
            )
            bw = singles.tile([1, H_DIM], BF16)
            nc.vector.tensor_mul(bw, bq_sb, wtb[:1, :])
            ones_row = singles.tile([1, P], BF16)
            nc.vector.memset(ones_row, 1.0)

        # w_q transposed + scaled by weight_tensor: wqTs [4x128, 120] bf16
        wqTs = singles.tile([P, ND * H_DIM], BF16)

        # persistent normalized-transposed context [120 (pad 128), S] bf16
        ctxT = singles.tile([P, S], BF16)

        with ExitStack() as ph0:
            ps_w = ph0.enter_context(
                tc.tile_pool(name="ps_w", bufs=2, space="PSUM"))
            for c in range(ND):
                wqT_ps = ps_w.tile([P, H_DIM], BF16)
                nc.tensor.transpose(
                    wqT_ps, wq_sb[:, c * P:(c + 1) * P], ident[:H_DIM, :H_DIM])
                # evict + fold in weight_tensor scale
                nc.vector.tensor_mul(
                    wqTs[:, c * H_DIM:(c + 1) * H_DIM], wqT_ps, wtb)

        with ExitStack() as ph1:
            # ---------- Phase 1: build ctxT ----------
            tmp_p = ph1.enter_context(tc.tile_pool(name="ph1tmp", bufs=2))
            st_p = ph1.enter_context(tc.tile_pool(name="ph1st", bufs=2))
            ps_c = ph1.enter_context(
                tc.tile_pool(name="ps_c", bufs=3, space="PSUM"))
            ps_ct = ph1.enter_context(
                tc.tile_pool(name="ps_ct", bufs=2, space="PSUM"))

            for i in range(NT):
                s0 = i * P
                # context tile [s=128, k=120] = q @ (w_q * wt).T (+ b*wt)
                ctx_ps = ps_c.tile([P, H_DIM], F32)
                for c in range(ND):
                    nc.tensor.matmul(
                        ctx_ps,
                        lhsT=qT[c][:, s0:s0 + P],
                        rhs=wqTs[:, c * H_DIM:(c + 1) * H_DIM],
                        start=(c == 0),
                        stop=(c == ND - 1 and not add_bias))
                if add_bias:
                    nc.tensor.matmul(
                        ctx_ps, lhsT=ones_row, rhs=bw, start=False, stop=True)

                # row L2 norm^2: ACT Square with free-dim accumulate
                sq = tmp_p.tile([P, H_DIM], F32, tag="sq")
                nsq = st_p.tile([P, 1], F32, tag="nsq")
                nc.scalar.activation(sq, ctx_ps, Act.Square, accum_out=nsq)

                # norm = sqrt(nsq), one Newton step, then rstd = 2/(s0+x/s0)
                sroot = st_p.tile([P, 1], F32, tag="sroot")
                nc.scalar.activation(sroot, nsq, Act.Sqrt)
                r0 = st_p.tile([P, 1], F32, tag="r0")
                nc.vector.reciprocal(r0, sroot)
                t1 = st_p.tile([P, 1], F32, tag="t1")
                nc.vector.tensor_mul(t1, nsq, r0)
                ssum = st_p.tile([P, 1], F32, tag="ssum")
                nc.vector.tensor_add(ssum, sroot, t1)
                nc.vector.tensor_scalar_max(ssum, ssum, 2e-12)
                u = st_p.tile([P, 1], F32, tag="u")
                nc.vector.reciprocal(u, ssum)
                rstd = st_p.tile([P, 1], F32, tag="rstd")
                nc.vector.tensor_scalar_mul(rstd, u, 2.0)

                # normalize + evict to bf16: ctx_n [s, k]
                ctxn = tmp_p.tile([P, H_DIM], BF16, tag="ctxn")
                nc.scalar.activation(ctxn, ctx_ps, Act.Copy, scale=rstd)

                # transpose to [k, s] and park into ctxT
                ctxT_ps = ps_ct.tile([H_DIM, P], BF16)
                nc.tensor.transpose(ctxT_ps, ctxn, ident)
                if i % 2 == 0:
                    nc.vector.tensor_copy(ctxT[:H_DIM, s0:s0 + P], ctxT_ps)
                else:
                    nc.scalar.copy(ctxT[:H_DIM, s0:s0 + P], ctxT_ps)

        # ---------- Phase 2: scores + masked softmax ----------
        with ExitStack() as ph2:
            mask_p = ph2.enter_context(
                tc.tile_pool(name="maskp", bufs=CFG["mask_bufs"]))
            out_p = ph2.enter_context(
                tc.tile_pool(name="outp", bufs=CFG["out_bufs"]))
            ech_p = ph2.enter_context(
                tc.tile_pool(name="echp", bufs=CFG["ech_bufs"]))
            sum_p = ph2.enter_context(tc.tile_pool(name="sump", bufs=3))
            ps2 = ph2.enter_context(
                tc.tile_pool(name="ps2", bufs=CFG["ps2_bufs"], space="PSUM"))

            for i in range(NT):
                q0 = i * P
                mask_sb = mask_p.tile([P, S], U8)
                nc.sync.dma_start(out=mask_sb, in_=m_dram[q0:q0 + P, :])

                out_sb = out_p.tile([P, S], BF16)
                sums = sum_p.tile([P, NCH], F32, tag="sums")
                lhsT = ctxT[:H_DIM, q0:q0 + P]
                for j in range(NCH):
                    c0 = j * 512
                    sc_ps = ps2.tile([P, 512], F32)
                    nc.tensor.matmul(
                        sc_ps, lhsT=lhsT, rhs=ctxT[:H_DIM, c0:c0 + 512],
                        start=True, stop=True)
                    # exp: ACT drains PSUM -> SBUF bf16
                    ech = ech_p.tile([P, 512], BF16)
                    nc.scalar.activation(ech, sc_ps, Act.Exp)
                    # mask multiply + row-sum partial on DVE (all operands
                    # SBUF -> 2x mode): out = (ech * 1.0) * mask, accum = sum
                    nc.vector.scalar_tensor_tensor(
                        out=out_sb[:, c0:c0 + 512],
                        in0=ech,
                        scalar=1.0,
                        in1=mask_sb[:, c0:c0 + 512],
                        op0=Alu.mult,
                        op1=Alu.mult,
                        accum_out=sums[:, j:j + 1])

                # reduce the NCH partials (op1=bypass passes in0 through)
                scr = sum_p.tile([P, NCH], F32, tag="scr")
                tot = sum_p.tile([P, 1], F32, tag="tot")
                nc.vector.scalar_tensor_tensor(
                    out=scr, in0=sums, scalar=1.0, in1=sums,
                    op0=Alu.mult, op1=Alu.bypass, accum_out=tot)
                rden = sum_p.tile([P, 1], F32, tag="rden")
                nc.vector.reciprocal(rden, tot)

                # normalize in place (bf16), then store via the ACT HWDGE ring
                for h in range(CFG["scale_split"]):
                    c0 = h * SSP
                    nc.vector.tensor_scalar_mul(
                        out_sb[:, c0:c0 + SSP], out_sb[:, c0:c0 + SSP], rden)
                    nc.scalar.dma_start(
                        out=out_dram[q0:q0 + P, c0:c0 + SSP],
                        in_=out_sb[:, c0:c0 + SSP])

    nc.compile()
    return nc


def _run(nc, in_maps, trace=False, tmpdir=None):
    from concourse import bass_utils
    return bass_utils.run_bass_kernel_spmd(
        nc, in_maps, core_ids=list(range(len(in_maps))), trace=trace,
        tmpdir=tmpdir)


def prepare(inputs):
    """Host-side marshalling: shard over batch, recode dtypes/layout."""
    query = np.asarray(inputs["query"], np.float32)
    mask = np.asarray(inputs["mask"])
    w_q = np.asarray(inputs["w_q"], np.float32)
    b_q = np.asarray(inputs["b_q"], np.float32)
    wt = np.ascontiguousarray(
        np.asarray(inputs["weight_tensor"], np.float32).reshape(1, H_DIM))

    B, S, _ = query.shape
    assert B == N_CORES
    qT = np.transpose(query, (0, 2, 1)).astype(ml_dtypes.bfloat16)
    mask_u8 = mask.astype(np.uint8)
    wq_bf = w_q.astype(ml_dtypes.bfloat16)
    add_bias = bool(np.any(b_q))

    in_maps = []
    for b in range(B):
        m = dict(qT=np.ascontiguousarray(qT[b]),
                 mask=np.ascontiguousarray(mask_u8[b]),
                 w_q=wq_bf, weight_tensor=wt)
        if add_bias:
            m["b_q"] = b_q
        in_maps.append(m)
    return in_maps, S, add_bias


def kernel(**inputs: np.ndarray) -> np.ndarray:
    in_maps, S, add_bias = prepare(inputs)
    nc = build_nc(S, add_bias=add_bias)
    res = _run(nc, in_maps)
    out = np.stack(
        [res.results[b]["out"] for b in range(len(in_maps))], axis=0)
    return out.astype(np.float32)
